# revision 5
# baseline (speedup 1.0000x reference)
"""Trainium2 Bass kernel for a 2-layer feed-forward LIF recurrence.

Reference semantics (per time step, two stacked LIF cells, f32):
    vd = v + 0.2*(i - v);  id = i + 0.4*(-i)
    z  = (vd > 1);         v' = (1 - z) * vd;   i' = id + inp
layer1 input = x_t, layer2 input = z1_t, output = z2_t.

Rescaled state: U = 5*v, I = i (raw). Then
    y  = 0.8*U + I;  z = (y > 5);  U' = (1-z)*y;  I' = 0.6*I + inp
with NO prescaling of x needed anywhere.

Core trick: one fused custom-DVE op does decay+add+threshold+reset in a
single pass, writing FLT_MIN (-3.4e38) as a spike *sentinel* instead of 0:
    U' = select(0.8*(U*(U > -1e38)) + I > 5,  -FLT_MAX,  0.8*(U*(U>-1e38)) + I)
The (U > -1e38) factor lazily cleans last step's sentinel back to 0.
Spikes are then recovered with cheap compares: z = (U' < -1e38), which runs
at the DVE's 2x tensor-scalar rate; layer-2 z2 extraction is done in bulk
per 8-step block on the otherwise-idle Activation engine (Sign+Relu), with
layer-2's voltage state living directly in the staging buffer.

Engine budget per step (per core, [128 x 256] per layer):
  DVE : A1 (custom, U1) 327 + B (z1 ts 2x) 194 + A2 (custom, U2) 327 = 848
  Pool: C1 (stt I1 += x) 451 + C2 (stt I2 += z1) 451              = 902
  ACT : bulk z2 = relu(sign(-U2 - 1e38))  amortized               = 473
C1/C2 are split (not one 512-wide stt) so the I1 recurrence never waits on
z1, keeping the A1->B->C2 chain off the critical cycle.

Sharding: data-parallel over batch. B=16 -> 2 batches per core across 8
NeuronCores; T=256 scan runs on-chip with state resident in SBUF.
"""
import numpy as np

import concourse.bass as bass
import concourse.bacc as bacc
import concourse.tile as tile
from concourse import mybir
from concourse.bass_utils import run_bass_kernel_spmd
from concourse.dve_ops import (
    DveOp,
    OPS,
    CUSTOM_DVE_SPECS,
    _SUB_OPCODE_FOR_NAME,
    _CUSTOM_DVE_ROW_BASE,
)
from concourse.dve_spec import Spec, Src0, Src1, C0, C1, C2, MaxNeg, select, lower
from concourse.dve_uop import DveOpSpec

T, B, H, W = 256, 16, 128, 128
NCORES = 8
BPC = B // NCORES            # batches per core
P = 128                      # SBUF partitions
F = (BPC * H * W) // P       # 256 free elems per layer per step
TBLK = 8                     # time steps per staging block

F32 = mybir.dt.float32
OP = mybir.AluOpType
AF = mybir.ActivationFunctionType

DEC_V = float(np.float32(1.0) - np.float32(1e-3 * 200.0))  # 0.8
DEC_I = float(np.float32(1.0) - np.float32(1e-3 * 400.0))  # 0.6
VTH = 5.0                    # threshold in U = 5*v scale
SENT_THR = -1e38             # anything below this is a spike sentinel
FMIN = float(np.finfo(np.float32).min)


def _ref_lif(in0, in1, s0, s1, imm2):
    """CoreSim reference for LIF_FUSED_ANT: in0=U, in1=I, s0=decay,
    s1=threshold, imm2=sentinel-detect bound."""
    ind = (imm2 < in0).astype(np.float32)
    y = ((in0.astype(np.float32) * ind) * s0 + in1).astype(np.float32)
    return np.where(s1 < y, np.float32(FMIN), y).astype(np.float32)


def _register_lif_op():
    ind = C2 < Src0                      # 0 if sentinel, 1 otherwise
    y = (Src0 * ind) * C0 + Src1         # decayed voltage + synaptic current
    spec = Spec(body=select(C1 < y, MaxNeg, y), reference=_ref_lif)
    shas = {}
    for ver in ("v3", "v4"):
        try:
            shas[ver] = DveOpSpec(
                name="LIF_FUSED_ANT", opcode=1, uops=lower(spec, ver=ver),
                rd1_en=True,
            ).sha(ver)
        except ValueError:
            pass
    op = DveOp("LIF_FUSED_ANT", spec, subdim=False, uops_sha=shas)
    if op.name not in _SUB_OPCODE_FOR_NAME:
        OPS.append(op)
        CUSTOM_DVE_SPECS[op.name] = op.spec
        _SUB_OPCODE_FOR_NAME[op.name] = _CUSTOM_DVE_ROW_BASE + len(OPS) - 1
    return op


LIF = _register_lif_op()


def build_nc():
    nc = bacc.Bacc("TRN2")
    x_d = nc.declare_dram_parameter("x", [T, P, F], F32, isOutput=False)
    o_d = nc.declare_dram_parameter("out", [T, P, F], F32, isOutput=True)

    with tile.TileContext(nc) as tc:
        with (
            tc.tile_pool(name="state", bufs=1) as sp,
            tc.tile_pool(name="io", bufs=3) as iop,
        ):
            U1 = sp.tile([P, F], F32, tag="U1")
            IA = sp.tile([P, 2 * F], F32, tag="IA")   # [I1 | I2], parity 0
            IB = sp.tile([P, 2 * F], F32, tag="IB")   # [I1 | I2], parity 1
            UBOOT = sp.tile([P, F], F32, tag="UBOOT")
            BIASN = sp.tile([P, 1], F32, tag="BIASN")  # Sign bias: -1e38
            nc.vector.memset(U1[:], 0.0)
            nc.vector.memset(IA[:], 0.0)
            nc.vector.memset(IB[:], 0.0)
            nc.gpsimd.memset(UBOOT[:], 0.0)
            nc.gpsimd.memset(BIASN[:], -1e38)

            u2prev = UBOOT[:]
            for t0 in range(0, T, TBLK):
                # per step k: [x_t (F) | z1_t (F)] so the I updates read
                # contiguous slices and DMA strides over the x slots.
                XB = iop.tile([P, TBLK * 2 * F], F32, tag="xb")
                UB = iop.tile([P, TBLK * F], F32, tag="ub")  # staged U2'
                ZB = iop.tile([P, TBLK * F], F32, tag="zb")  # z2 out block
                nc.sync.dma_start(
                    XB[:].rearrange("p (t two f) -> p t two f",
                                    t=TBLK, two=2)[:, :, 0, :],
                    x_d[t0 : t0 + TBLK].rearrange("t p f -> p t f"),
                )
                for k in range(TBLK):
                    t = t0 + k
                    Icur = (IA, IB)[t % 2]
                    Inxt = (IA, IB)[(t + 1) % 2]
                    xs = XB[:, bass.ts(2 * k, F)]
                    z1s = XB[:, bass.ts(2 * k + 1, F)]
                    u2slot = UB[:, bass.ts(k, F)]
                    # A1: U1 <- fused decay/add/threshold/reset (in place)
                    nc.vector._custom_dve(
                        LIF, out=U1[:], in0=U1[:], in1=Icur[:, :F],
                        s0=DEC_V, s1=VTH, imm2=SENT_THR,
                    )
                    # B: z1 = (U1 < -1e38) in {0.0, 1.0}
                    nc.vector.tensor_scalar(z1s, U1[:], SENT_THR, None, OP.is_lt)
                    # A2: staged U2' <- fused step (state lives in UB slots)
                    nc.vector._custom_dve(
                        LIF, out=u2slot, in0=u2prev, in1=Icur[:, F:],
                        s0=DEC_V, s1=VTH, imm2=SENT_THR,
                    )
                    # C1/C2 (Pool): synaptic currents, double-buffered
                    nc.gpsimd.scalar_tensor_tensor(
                        Inxt[:, :F], Icur[:, :F], DEC_I, xs, OP.mult, OP.add)
                    nc.gpsimd.scalar_tensor_tensor(
                        Inxt[:, F:], Icur[:, F:], DEC_I, z1s, OP.mult, OP.add)
                    u2prev = u2slot
                # D (ACT, bulk): z2 = relu(sign(-U2' - 1e38)) in {0.0, 1.0}
                nc.scalar.activation(ZB[:], UB[:], AF.Sign, bias=BIASN[:],
                                     scale=-1.0)
                nc.scalar.activation(ZB[:], ZB[:], AF.Relu)
                # out-DMA issued from the ACT queue: SP's sequencer is held
                # for the whole DMA (incl. transfer + sem-prop) in the cost
                # model, so splitting in/out across two queues de-serializes.
                nc.scalar.dma_start(
                    o_d[t0 : t0 + TBLK].rearrange("t p f -> p t f"),
                    ZB[:].rearrange("p (t f) -> p t f", t=TBLK),
                )
    nc.compile()
    return nc


_NC_CACHE = {}


def _get_nc():
    if "nc" not in _NC_CACHE:
        _NC_CACHE["nc"] = build_nc()
    return _NC_CACHE["nc"]


def _shard_inputs(x):
    shards = []
    for c in range(NCORES):
        xs = np.ascontiguousarray(x[:, c * BPC : (c + 1) * BPC]).reshape(T, P, F)
        shards.append({"x": xs})
    return shards


def _unshard(outs):
    parts = [o.reshape(T, BPC, H, W) for o in outs]
    return np.concatenate(parts, axis=1)


def kernel(x, _trace=False):
    x = np.asarray(x)
    assert x.shape == (T, B, H, W), x.shape
    nc = _get_nc()
    res = run_bass_kernel_spmd(nc, _shard_inputs(x), list(range(NCORES)),
                               trace=_trace)
    out = _unshard([np.asarray(r["out"]) for r in res.results])
    if _trace:
        return out.astype(np.float32), res
    return out.astype(np.float32)


# revision 8
# speedup vs baseline: 1.0409x; 1.0409x over previous
"""Trainium2 Bass kernel for a 2-layer feed-forward LIF recurrence.

Reference semantics (per time step, two stacked LIF cells, f32):
    vd = v + 0.2*(i - v);  id = i + 0.4*(-i)
    z  = (vd > 1);         v' = (1 - z) * vd;   i' = id + inp
layer1 input = x_t, layer2 input = z1_t, output = z2_t.

Rescaled state: U = 5*v, I = i (raw). Then
    y  = 0.8*U + I;  z = (y > 5);  U' = (1-z)*y;  I' = 0.6*I + inp
with NO prescaling of x needed anywhere.

Core trick: one fused custom-DVE op does decay+add+threshold+reset in a
single pass, writing FLT_MIN (-3.4e38) as a spike *sentinel* instead of 0:
    U' = select(0.8*(U*(U > -1e38)) + I > 5,  -FLT_MAX,  0.8*(U*(U>-1e38)) + I)
The (U > -1e38) factor lazily cleans last step's sentinel back to 0.
Spikes are then recovered with cheap compares: z = (U' < -1e38), which runs
at the DVE's 2x tensor-scalar rate; layer-2 z2 extraction is done in bulk
per 8-step block on the otherwise-idle Activation engine (Sign+Relu), with
layer-2's voltage state living directly in the staging buffer.

Engine budget per step (per core, [128 x 256] per layer):
  DVE : A1 (custom, U1) 327 + B (z1 ts 2x) 194 + A2 (custom, U2) 327 = 848
  Pool: C1 (stt I1 += x) 451 + C2 (stt I2 += z1) 451              = 902
  ACT : bulk z2 = relu(sign(-U2 - 1e38))  amortized               = 473
C1/C2 are split (not one 512-wide stt) so the I1 recurrence never waits on
z1, keeping the A1->B->C2 chain off the critical cycle.

Sharding: data-parallel over batch. B=16 -> 2 batches per core across 8
NeuronCores; T=256 scan runs on-chip with state resident in SBUF.
"""
import numpy as np

import concourse.bass as bass
import concourse.bacc as bacc
import concourse.tile as tile
from concourse import mybir
from concourse.bass_utils import run_bass_kernel_spmd
from concourse.dve_ops import (
    DveOp,
    OPS,
    CUSTOM_DVE_SPECS,
    _SUB_OPCODE_FOR_NAME,
    _CUSTOM_DVE_ROW_BASE,
)
from concourse.dve_spec import Spec, Src0, Src1, C0, C1, C2, MaxNeg, select, lower
from concourse.dve_uop import DveOpSpec
from concourse.tile_rust import add_dep_helper

T, B, H, W = 256, 16, 128, 128
NCORES = 8
BPC = B // NCORES            # batches per core
P = 128                      # SBUF partitions
F = (BPC * H * W) // P       # 256 free elems per layer per step
TBLK = 8                     # time steps per staging block

F32 = mybir.dt.float32
OP = mybir.AluOpType
AF = mybir.ActivationFunctionType

DEC_V = float(np.float32(1.0) - np.float32(1e-3 * 200.0))  # 0.8
DEC_I = float(np.float32(1.0) - np.float32(1e-3 * 400.0))  # 0.6
VTH = 5.0                    # threshold in U = 5*v scale
SENT_THR = -1e38             # anything below this is a spike sentinel
FMIN = float(np.finfo(np.float32).min)


def _ref_lif(in0, in1, s0, s1, imm2):
    """CoreSim reference for LIF_FUSED_ANT: in0=U, in1=I, s0=decay,
    s1=threshold, imm2=sentinel-detect bound."""
    ind = (imm2 < in0).astype(np.float32)
    y = ((in0.astype(np.float32) * ind) * s0 + in1).astype(np.float32)
    return np.where(s1 < y, np.float32(FMIN), y).astype(np.float32)


def _register_lif_op():
    ind = C2 < Src0                      # 0 if sentinel, 1 otherwise
    y = (Src0 * ind) * C0 + Src1         # decayed voltage + synaptic current
    spec = Spec(body=select(C1 < y, MaxNeg, y), reference=_ref_lif)
    shas = {}
    for ver in ("v3", "v4"):
        try:
            shas[ver] = DveOpSpec(
                name="LIF_FUSED_ANT", opcode=1, uops=lower(spec, ver=ver),
                rd1_en=True,
            ).sha(ver)
        except ValueError:
            pass
    op = DveOp("LIF_FUSED_ANT", spec, subdim=False, uops_sha=shas)
    if op.name not in _SUB_OPCODE_FOR_NAME:
        OPS.append(op)
        CUSTOM_DVE_SPECS[op.name] = op.spec
        _SUB_OPCODE_FOR_NAME[op.name] = _CUSTOM_DVE_ROW_BASE + len(OPS) - 1
    return op


LIF = _register_lif_op()


def build_nc():
    nc = bacc.Bacc("TRN2")
    x_d = nc.declare_dram_parameter("x", [T, P, F], F32, isOutput=False)
    o_d = nc.declare_dram_parameter("out", [T, P, F], F32, isOutput=True)

    with tile.TileContext(nc) as tc:
        with (
            tc.tile_pool(name="state", bufs=1) as sp,
            tc.tile_pool(name="io", bufs=3) as iop,
        ):
            U1 = sp.tile([P, F], F32, tag="U1")
            IA = sp.tile([P, 2 * F], F32, tag="IA")   # [I1 | I2], parity 0
            IB = sp.tile([P, 2 * F], F32, tag="IB")   # [I1 | I2], parity 1
            UBOOT = sp.tile([P, F], F32, tag="UBOOT")
            BIASN = sp.tile([P, 1], F32, tag="BIASN")  # Sign bias: -1e38
            nc.vector.memset(U1[:], 0.0)
            nc.vector.memset(IA[:], 0.0)
            nc.vector.memset(IB[:], 0.0)
            nc.gpsimd.memset(UBOOT[:], 0.0)
            nc.gpsimd.memset(BIASN[:], -1e38)

            u2prev = UBOOT[:]
            for t0 in range(0, T, TBLK):
                # per step k: [x_t (F) | z1_t (F)] so the I updates read
                # contiguous slices and DMA strides over the x slots.
                XB = iop.tile([P, TBLK * 2 * F], F32, tag="xb")
                UB = iop.tile([P, TBLK * F], F32, tag="ub")  # staged U2'
                ZB = iop.tile([P, TBLK * F], F32, tag="zb")  # z2 out block
                nc.sync.dma_start(
                    XB[:].rearrange("p (t two f) -> p t two f",
                                    t=TBLK, two=2)[:, :, 0, :],
                    x_d[t0 : t0 + TBLK].rearrange("t p f -> p t f"),
                )
                for k in range(TBLK):
                    t = t0 + k
                    Icur = (IA, IB)[t % 2]
                    Inxt = (IA, IB)[(t + 1) % 2]
                    xs = XB[:, bass.ts(2 * k, F)]
                    z1s = XB[:, bass.ts(2 * k + 1, F)]
                    u2slot = UB[:, bass.ts(k, F)]
                    # A1: U1 <- fused decay/add/threshold/reset (in place)
                    nc.vector._custom_dve(
                        LIF, out=U1[:], in0=U1[:], in1=Icur[:, :F],
                        s0=DEC_V, s1=VTH, imm2=SENT_THR,
                    )
                    # B: z1 = (U1 < -1e38) in {0.0, 1.0}
                    b_inst = nc.vector.tensor_scalar(
                        z1s, U1[:], SENT_THR, None, OP.is_lt)
                    # A2: staged U2' <- fused step (state lives in UB slots)
                    a2_inst = nc.vector._custom_dve(
                        LIF, out=u2slot, in0=u2prev, in1=Icur[:, F:],
                        s0=DEC_V, s1=VTH, imm2=SENT_THR,
                    )
                    # Keep B ahead of A2 in the DVE queue: the binding cycle
                    # is B -> C2 -> A2(next) -> B; A2 slotting first adds its
                    # latency to that loop.
                    add_dep_helper(a2_inst.ins, b_inst.ins,
                                   reason="schedule z1 extraction before A2")
                    # C1/C2 (Pool): synaptic currents, double-buffered
                    nc.gpsimd.scalar_tensor_tensor(
                        Inxt[:, :F], Icur[:, :F], DEC_I, xs, OP.mult, OP.add)
                    nc.gpsimd.scalar_tensor_tensor(
                        Inxt[:, F:], Icur[:, F:], DEC_I, z1s, OP.mult, OP.add)
                    u2prev = u2slot
                # D (ACT, bulk): z2 = relu(sign(-U2' - 1e38)) in {0.0, 1.0}
                nc.scalar.activation(ZB[:], UB[:], AF.Sign, bias=BIASN[:],
                                     scale=-1.0)
                nc.scalar.activation(ZB[:], ZB[:], AF.Relu)
                # out-DMA issued from the ACT queue: SP's sequencer is held
                # for the whole DMA (incl. transfer + sem-prop) in the cost
                # model, so splitting in/out across two queues de-serializes.
                nc.scalar.dma_start(
                    o_d[t0 : t0 + TBLK].rearrange("t p f -> p t f"),
                    ZB[:].rearrange("p (t f) -> p t f", t=TBLK),
                )
    nc.compile()
    return nc


_NC_CACHE = {}


def _get_nc():
    if "nc" not in _NC_CACHE:
        _NC_CACHE["nc"] = build_nc()
    return _NC_CACHE["nc"]


def _shard_inputs(x):
    shards = []
    for c in range(NCORES):
        xs = np.ascontiguousarray(x[:, c * BPC : (c + 1) * BPC]).reshape(T, P, F)
        shards.append({"x": xs})
    return shards


def _unshard(outs):
    parts = [o.reshape(T, BPC, H, W) for o in outs]
    return np.concatenate(parts, axis=1)


def kernel(x, _trace=False):
    x = np.asarray(x)
    assert x.shape == (T, B, H, W), x.shape
    nc = _get_nc()
    res = run_bass_kernel_spmd(nc, _shard_inputs(x), list(range(NCORES)),
                               trace=_trace)
    out = _unshard([np.asarray(r["out"]) for r in res.results])
    if _trace:
        return out.astype(np.float32), res
    return out.astype(np.float32)


# revision 9
# speedup vs baseline: 1.2410x; 1.1922x over previous
"""Trainium2 Bass kernel for a 2-layer feed-forward LIF recurrence.

Reference semantics (per time step, two stacked LIF cells, f32):
    vd = v + 0.2*(i - v);  id = i + 0.4*(-i)
    z  = (vd > 1);         v' = (1 - z) * vd;   i' = id + inp
layer1 input = x_t, layer2 input = z1_t, output = z2_t.

Rescaled state: U = 5*v, I = i (raw). Then
    y  = 0.8*U + I;  z = (y > 5);  U' = (1-z)*y;  I' = 0.6*I + inp
with NO prescaling of x needed anywhere.

Core trick: one fused custom-DVE op does decay+add+threshold+reset in a
single pass, writing FLT_MIN (-3.4e38) as a spike *sentinel* instead of 0:
    U' = select(0.8*(U*(U > -1e38)) + I > 5,  -FLT_MAX,  0.8*(U*(U>-1e38)) + I)
The (U > -1e38) factor lazily cleans last step's sentinel back to 0.
Spikes are then recovered with cheap compares: z = (U' < -1e38), which runs
at the DVE's 2x tensor-scalar rate; layer-2 z2 extraction is done in bulk
per 8-step block on the otherwise-idle Activation engine (Sign+Relu), with
layer-2's voltage state living directly in the staging buffer.

Engine budget per step (per core, [128 x 256] per layer):
  DVE : A1 (custom, U1) 327 + B (z1 ts 2x) 194 + A2 (custom, U2) 327 = 848
  Pool: C1 (stt I1 += x) 451 + C2 (stt I2 += z1) 451              = 902
  ACT : bulk z2 = relu(sign(-U2 - 1e38))  amortized               = 473
C1/C2 are split (not one 512-wide stt) so the I1 recurrence never waits on
z1, keeping the A1->B->C2 chain off the critical cycle.

Sharding: data-parallel over batch. B=16 -> 2 batches per core across 8
NeuronCores; T=256 scan runs on-chip with state resident in SBUF.
"""
import numpy as np

import concourse.bass as bass
import concourse.bacc as bacc
import concourse.tile as tile
from concourse import mybir
from concourse.bass_utils import run_bass_kernel_spmd
from concourse.dve_ops import (
    DveOp,
    OPS,
    CUSTOM_DVE_SPECS,
    _SUB_OPCODE_FOR_NAME,
    _CUSTOM_DVE_ROW_BASE,
)
from concourse.dve_spec import Spec, Src0, Src1, C0, C1, C2, MaxNeg, select, lower
from concourse.dve_uop import DveOpSpec
from concourse.tile_rust import add_dep_helper

T, B, H, W = 256, 16, 128, 128
NCORES = 8
BPC = B // NCORES            # batches per core
P = 128                      # SBUF partitions
F = (BPC * H * W) // P       # 256 free elems per layer per step
TBLK = 8                     # time steps per staging block

F32 = mybir.dt.float32
OP = mybir.AluOpType
AF = mybir.ActivationFunctionType

DEC_V = float(np.float32(1.0) - np.float32(1e-3 * 200.0))  # 0.8
DEC_I = float(np.float32(1.0) - np.float32(1e-3 * 400.0))  # 0.6
VTH = 5.0                    # threshold in U = 5*v scale
SENT_THR = -1e38             # anything below this is a spike sentinel
FMIN = float(np.finfo(np.float32).min)


def _ref_lif(in0, in1, s0, s1, imm2):
    """CoreSim reference for LIF_FUSED_ANT: in0=U, in1=I, s0=decay,
    s1=threshold, imm2=sentinel-detect bound."""
    ind = (imm2 < in0).astype(np.float32)
    y = ((in0.astype(np.float32) * ind) * s0 + in1).astype(np.float32)
    return np.where(s1 < y, np.float32(FMIN), y).astype(np.float32)


def _register_lif_op():
    ind = C2 < Src0                      # 0 if sentinel, 1 otherwise
    y = (Src0 * ind) * C0 + Src1         # decayed voltage + synaptic current
    spec = Spec(body=select(C1 < y, MaxNeg, y), reference=_ref_lif)
    shas = {}
    for ver in ("v3", "v4"):
        try:
            shas[ver] = DveOpSpec(
                name="LIF_FUSED_ANT", opcode=1, uops=lower(spec, ver=ver),
                rd1_en=True,
            ).sha(ver)
        except ValueError:
            pass
    op = DveOp("LIF_FUSED_ANT", spec, subdim=False, uops_sha=shas)
    if op.name not in _SUB_OPCODE_FOR_NAME:
        OPS.append(op)
        CUSTOM_DVE_SPECS[op.name] = op.spec
        _SUB_OPCODE_FOR_NAME[op.name] = _CUSTOM_DVE_ROW_BASE + len(OPS) - 1
    return op


LIF = _register_lif_op()


def build_nc():
    nc = bacc.Bacc("TRN2")
    x_d = nc.declare_dram_parameter("x", [T, P, F], F32, isOutput=False)
    o_d = nc.declare_dram_parameter("out", [T, P, F], F32, isOutput=True)

    with tile.TileContext(nc) as tc:
        with (
            tc.tile_pool(name="state", bufs=1) as sp,
            tc.tile_pool(name="io", bufs=3) as iop,
        ):
            U1 = sp.tile([P, F], F32, tag="U1")
            IA = sp.tile([P, 2 * F], F32, tag="IA")   # [I1 | I2], parity 0
            IB = sp.tile([P, 2 * F], F32, tag="IB")   # [I1 | I2], parity 1
            UBOOT = sp.tile([P, F], F32, tag="UBOOT")
            BIASN = sp.tile([P, 1], F32, tag="BIASN")  # Sign bias: -1e38
            nc.vector.memset(U1[:], 0.0)
            nc.vector.memset(IA[:], 0.0)
            nc.vector.memset(IB[:], 0.0)
            nc.gpsimd.memset(UBOOT[:], 0.0)
            nc.gpsimd.memset(BIASN[:], -1e38)

            u2prev = UBOOT[:]
            for t0 in range(0, T, TBLK):
                # per step k: [x_t (F) | z1_t (F)] so the I updates read
                # contiguous slices and DMA strides over the x slots.
                XB = iop.tile([P, TBLK * 2 * F], F32, tag="xb")
                UB = iop.tile([P, TBLK * F], F32, tag="ub")  # staged U2'
                ZB = iop.tile([P, TBLK * F], F32, tag="zb")  # z2 out block
                nc.sync.dma_start(
                    XB[:].rearrange("p (t two f) -> p t two f",
                                    t=TBLK, two=2)[:, :, 0, :],
                    x_d[t0 : t0 + TBLK].rearrange("t p f -> p t f"),
                )
                for k in range(TBLK):
                    t = t0 + k
                    Icur = (IA, IB)[t % 2]
                    Inxt = (IA, IB)[(t + 1) % 2]
                    xs = XB[:, bass.ts(2 * k, F)]
                    z1s = XB[:, bass.ts(2 * k + 1, F)]
                    u2slot = UB[:, bass.ts(k, F)]
                    # A1: U1 <- fused decay/add/threshold/reset (in place)
                    nc.vector._custom_dve(
                        LIF, out=U1[:], in0=U1[:], in1=Icur[:, :F],
                        s0=DEC_V, s1=VTH, imm2=SENT_THR,
                    )
                    # B: z1 = (U1 < -1e38) in {0.0, 1.0}
                    b_inst = nc.vector.tensor_scalar(
                        z1s, U1[:], SENT_THR, None, OP.is_lt)
                    # A2: staged U2' <- fused step (state lives in UB slots)
                    a2_inst = nc.vector._custom_dve(
                        LIF, out=u2slot, in0=u2prev, in1=Icur[:, F:],
                        s0=DEC_V, s1=VTH, imm2=SENT_THR,
                    )
                    # Keep B ahead of A2 in the DVE queue: the binding cycle
                    # is B -> C2 -> A2(next) -> B; A2 slotting first adds its
                    # latency to that loop.
                    add_dep_helper(a2_inst.ins, b_inst.ins, sync=False,
                                   reason="schedule z1 extraction before A2")
                    # C1/C2 (Pool): synaptic currents, double-buffered
                    nc.gpsimd.scalar_tensor_tensor(
                        Inxt[:, :F], Icur[:, :F], DEC_I, xs, OP.mult, OP.add)
                    nc.gpsimd.scalar_tensor_tensor(
                        Inxt[:, F:], Icur[:, F:], DEC_I, z1s, OP.mult, OP.add)
                    u2prev = u2slot
                # D (ACT, bulk): z2 = relu(sign(-U2' - 1e38)) in {0.0, 1.0}
                nc.scalar.activation(ZB[:], UB[:], AF.Sign, bias=BIASN[:],
                                     scale=-1.0)
                nc.scalar.activation(ZB[:], ZB[:], AF.Relu)
                # out-DMA issued from the ACT queue: SP's sequencer is held
                # for the whole DMA (incl. transfer + sem-prop) in the cost
                # model, so splitting in/out across two queues de-serializes.
                nc.scalar.dma_start(
                    o_d[t0 : t0 + TBLK].rearrange("t p f -> p t f"),
                    ZB[:].rearrange("p (t f) -> p t f", t=TBLK),
                )
    nc.compile()
    return nc


_NC_CACHE = {}


def _get_nc():
    if "nc" not in _NC_CACHE:
        _NC_CACHE["nc"] = build_nc()
    return _NC_CACHE["nc"]


def _shard_inputs(x):
    shards = []
    for c in range(NCORES):
        xs = np.ascontiguousarray(x[:, c * BPC : (c + 1) * BPC]).reshape(T, P, F)
        shards.append({"x": xs})
    return shards


def _unshard(outs):
    parts = [o.reshape(T, BPC, H, W) for o in outs]
    return np.concatenate(parts, axis=1)


def kernel(x, _trace=False):
    x = np.asarray(x)
    assert x.shape == (T, B, H, W), x.shape
    nc = _get_nc()
    res = run_bass_kernel_spmd(nc, _shard_inputs(x), list(range(NCORES)),
                               trace=_trace)
    out = _unshard([np.asarray(r["out"]) for r in res.results])
    if _trace:
        return out.astype(np.float32), res
    return out.astype(np.float32)
